# revision 20
# baseline (speedup 1.0000x reference)
"""Causal self-attention (quirky-reshape variant) on 8 TRN2 NeuronCores.

Key structural fact: the reference reshapes (B,S,H*dk) -> (B,H,S,dk) without a
transpose, so head h's Q/K/V come from rows [256h, 256h+256) of the [4096,1024]
projection output (reinterpreted as [4096,64]), and output rows [256h, 256h+256)
depend only on head h.  With 2 heads per core the problem is embarrassingly
parallel: core i consumes x rows [512i, 512i+512) + full weights and produces
output rows [512i, 512i+512).  No collectives.

Per-core pipeline (all matmuls in the "transposed-scores" orientation):
  qflat^T/kflat^T via o-stationary projection (host-permuted weights fold both
  the quirky reshape's d-extraction and a RoPE even/odd de-interleave into the
  PSUM partition order), vflat via r-stationary projection; RoPE as 3 full-width
  DVE passes; scores^T = K^T-stationary matmul (2 heads row-packed in the PE
  array); exp on ScalarE (scale=1/8 folded in, causal triangle masks added on
  PSUM, fully-masked columns skipped by ragged matmul widths); P@V with a
  [V|ones] stationary (ones column yields softmax denominators); normalize with
  reciprocal_approx_fast + a PE outer-product broadcast; output projection from
  strided-gathered concat^T tiles.
"""

import os

os.environ.setdefault("JAX_PLATFORMS", "cpu")

import numpy as np
import ml_dtypes

D = 1024          # d_model
H = 16            # heads
DK = 64           # head dim
S = 4096          # seq len
NC_N = 8          # cores
RPC = 512         # x rows per core
HPC = 2           # heads per core
NT_SK = 32        # sk tiles of 128 per head
ROPE_THETA = 10000.0
F32 = None        # set at build (mybir.dt.float32)
BF16 = None

_CACHE = {}


def _deint_perm():
    """o' -> o source index: within each 64-block, evens first then odds."""
    d_order = list(range(0, DK, 2)) + list(range(1, DK, 2))  # position d' -> d
    perm = np.zeros(D, dtype=np.int64)
    for c in range(H):
        for dp, d in enumerate(d_order):
            perm[c * DK + dp] = c * DK + d
    return perm


def _host_arrays(Wq, Wk, Wv, Wo):
    bf = ml_dtypes.bfloat16
    perm = _deint_perm()
    wqT = np.ascontiguousarray(Wq[perm, :].T).astype(bf)   # [in, o'] deint
    wkT = np.ascontiguousarray(Wk[perm, :].T).astype(bf)
    wvT = np.ascontiguousarray(Wv.T).astype(bf)            # [in, o] natural
    woT = np.ascontiguousarray(Wo.T).astype(bf)            # [o_c, o_out]

    # RoPE tables in the [Aev, Aod, Bev, Bod] partition grouping (32 rows each;
    # identical for both heads since the angle depends only on s).
    j = np.arange(0, DK, 2, dtype=np.float64) / DK
    inv_freq = 1.0 / (ROPE_THETA ** j)                     # [32]
    s = np.arange(S, dtype=np.float64)
    ang = np.outer(inv_freq, s)                            # [32, S]
    cos = np.cos(ang)
    sin = np.sin(ang)
    cs1 = np.concatenate([cos, cos, cos, cos], 0).astype(bf)       # mult RAW
    cs2 = np.concatenate([-sin, sin, -sin, sin], 0).astype(bf)     # mult SWAP
    # triangle mask: scores^T[p, q] valid iff p <= q (within a diagonal block)
    tri = np.where(
        np.arange(128)[:, None] <= np.arange(128)[None, :], 0.0, -1e30
    ).astype(np.float32)
    return wqT, wkT, wvT, woT, cs1, cs2, tri


def _build_program(dbg=False):
    import concourse.bass as bass
    import concourse.tile as tile
    from concourse import bacc, mybir

    f32 = mybir.dt.float32
    bf16 = mybir.dt.bfloat16
    EXP = mybir.ActivationFunctionType.Exp
    CPY = mybir.ActivationFunctionType.Copy

    nc = bacc.Bacc("TRN2", target_bir_lowering=False, debug=False,
                   num_devices=NC_N)

    xT = nc.dram_tensor("xT", [D, RPC], bf16, kind="ExternalInput").ap()
    wq = nc.dram_tensor("wqT", [D, D], bf16, kind="ExternalInput").ap()
    wk = nc.dram_tensor("wkT", [D, D], bf16, kind="ExternalInput").ap()
    wv = nc.dram_tensor("wvT", [D, D], bf16, kind="ExternalInput").ap()
    wo = nc.dram_tensor("woT", [D, D], bf16, kind="ExternalInput").ap()
    cs1d = nc.dram_tensor("cs1", [128, S], bf16, kind="ExternalInput").ap()
    cs2d = nc.dram_tensor("cs2", [128, S], bf16, kind="ExternalInput").ap()
    trid = nc.dram_tensor("tri", [128, 128], f32, kind="ExternalInput").ap()
    y = nc.dram_tensor("y", [RPC, D], f32, kind="ExternalOutput").ap()
    vfd = nc.dram_tensor("vflat_scratch", [RPC, D], bf16).ap()
    dbg_aps = {}
    if dbg:
        for nm, shp, dt in [
            ("dbg_qraw", [128, S], bf16), ("dbg_kraw", [128, S], bf16),
            ("dbg_qrot", [128, S], bf16), ("dbg_krot", [128, S], bf16),
            ("dbg_vsb0", [128, 65 * NT_SK], bf16),
            ("dbg_vsb1", [128, 65 * NT_SK], bf16),
            ("dbg_outT0", [64, S], bf16), ("dbg_outT1", [64, S], bf16),
            ("dbg_vflat", [RPC, D], bf16),
            ("dbg_outp0", [65, 512], f32), ("dbg_ps0", [128, 1536], f32),
            ("dbg_pch0", [128, 1536], bf16), ("dbg_dn0", [1, 512], f32),
            ("dbg_bc0", [64, 512], f32),
        ]:
            dbg_aps[nm] = nc.dram_tensor(nm, shp, dt, kind="ExternalOutput").ap()

    with tile.TileContext(nc) as tc:
        with (
            tc.tile_pool(name="big", bufs=3) as big,        # wq/wk/wv -> outTA/outTB/y_sb
            tc.tile_pool(name="wo", bufs=1) as wop,
            tc.tile_pool(name="xp", bufs=1) as xp,
            tc.tile_pool(name="qk", bufs=2) as qkp,          # qraw, kraw (become rot in place)
            tc.tile_pool(name="rope", bufs=4) as ropep,      # cs1, cs2, swQ, swK
            tc.tile_pool(name="vf", bufs=1) as vfp,
            tc.tile_pool(name="vsb", bufs=2) as vsbp,
            tc.tile_pool(name="mask", bufs=1) as maskp,
            tc.tile_pool(name="pp", bufs=3) as ppool,        # exp'd P chunks
            tc.tile_pool(name="ct", bufs=2) as ctp,          # concatT per (h, rt)
            tc.tile_pool(name="misc", bufs=1) as miscp,
        ):
            # ---------- phase 0: loads ----------
            xsb = xp.tile([128, 8 * RPC], bf16, tag="x")           # [p, kt*512+r]
            nc.sync.dma_start(xsb[:].rearrange("p (kt r) -> p kt r", kt=8),
                              xT.rearrange("(kt p) r -> p kt r", p=128))
            wq_sb = big.tile([128, 8 * D], bf16, tag="big")
            nc.sync.dma_start(wq_sb[:].rearrange("p (kt o) -> p kt o", kt=8),
                              wq.rearrange("(kt p) o -> p kt o", p=128))
            wk_sb = big.tile([128, 8 * D], bf16, tag="big")
            nc.sync.dma_start(wk_sb[:].rearrange("p (kt o) -> p kt o", kt=8),
                              wk.rearrange("(kt p) o -> p kt o", p=128))
            wv_sb = big.tile([128, 8 * D], bf16, tag="big")
            nc.sync.dma_start(wv_sb[:].rearrange("p (kt o) -> p kt o", kt=8),
                              wv.rearrange("(kt p) o -> p kt o", p=128))
            wo_sb = wop.tile([128, 8 * D], bf16, tag="wo")
            nc.sync.dma_start(wo_sb[:].rearrange("p (kt o) -> p kt o", kt=8),
                              wo.rearrange("(kt p) o -> p kt o", p=128))
            cs1_sb = ropep.tile([128, S], bf16, tag="rope")
            nc.sync.dma_start(cs1_sb[:], cs1d[:])
            cs2_sb = ropep.tile([128, S], bf16, tag="rope")
            nc.sync.dma_start(cs2_sb[:], cs2d[:])
            tri_sb = maskp.tile([128, 128], f32, tag="mask")
            nc.sync.dma_start(tri_sb[:], trid[:])

            misc = miscp.tile([128, 2048], f32, tag="misc")
            ones_row = misc[0:1, 0:64]
            nc.gpsimd.memset(ones_row, 1.0)

            # ---------- phase 1: projections ----------
            qraw = qkp.tile([128, S], bf16, tag="qk")   # [Aev,Aod,Bev,Bod] x s
            kraw = qkp.tile([128, S], bf16, tag="qk")

            with tc.tile_pool(name="psproj", bufs=3, space="PSUM") as psp:
                for w_sb, raw in ((wq_sb, qraw), (wk_sb, kraw)):
                    rv = raw[:].rearrange("p (r c) -> p r c", c=16)
                    for ot in range(8):
                        pq = psp.tile([128, RPC], f32, tag="ps")
                        for kt in range(8):
                            nc.tensor.matmul(
                                pq[:],
                                w_sb[:, kt * D + ot * 128: kt * D + ot * 128 + 128],
                                xsb[:, kt * RPC: (kt + 1) * RPC],
                                start=(kt == 0), stop=(kt == 7),
                            )
                        c0 = 2 * ot
                        # (A, c0): no partition shift -> ScalarE
                        nc.scalar.activation(rv[0:64, 0:256, c0], pq[0:64, 0:256], CPY)
                        # (B, c0): shift 0->64 -> DVE
                        nc.vector.tensor_copy(rv[64:128, 0:256, c0], pq[0:64, 256:512])
                        # (A, c0+1): shift 64->0 -> DVE
                        nc.vector.tensor_copy(rv[0:64, 0:256, c0 + 1], pq[64:128, 0:256])
                        # (B, c0+1): no shift -> ScalarE
                        nc.scalar.activation(rv[64:128, 0:256, c0 + 1], pq[64:128, 256:512], CPY)

                # V projection, r-stationary: vflat [r, o] natural
                vflat = vfp.tile([128, 4 * D], bf16, tag="vf")    # [p, rt*1024+o]
                for rt in range(4):
                    for ob in range(2):
                        pv = psp.tile([128, 512], f32, tag="ps")
                        for kt in range(8):
                            nc.tensor.matmul(
                                pv[:],
                                xsb[:, kt * RPC + rt * 128: kt * RPC + rt * 128 + 128],
                                wv_sb[:, kt * D + ob * 512: kt * D + ob * 512 + 512],
                                start=(kt == 0), stop=(kt == 7),
                            )
                        nc.scalar.activation(
                            vflat[:, rt * D + ob * 512: rt * D + ob * 512 + 512],
                            pv[:], CPY)

            # V reshape through DRAM: vflat [r,o] -> vsb_h[p, 65T+d] ([V|ones])
            nc.sync.dma_start(vfd.rearrange("(rt p) o -> p rt o", p=128),
                              vflat[:].rearrange("p (rt o) -> p rt o", rt=4))
            vsbs = []
            vld = vfd.rearrange("(h T a) (c d) -> h a c T d", h=2, T=32, a=8,
                                c=16, d=DK)
            for h in range(HPC):
                vsb = vsbp.tile([128, 65 * NT_SK], bf16, tag="vsb")
                nc.gpsimd.memset(vsb[:], 1.0)   # ones col at 65T+64 survives
                dstv = vsb[:].rearrange("(a c) (T d) -> a c T d", a=8, c=16,
                                        T=NT_SK, d=65)
                for a in range(8):
                    nc.sync.dma_start(dstv[a, :, :, 0:DK], vld[h, a])
                vsbs.append(vsb)

            if dbg:
                nc.sync.dma_start(dbg_aps["dbg_qraw"][:], qraw[:])
                nc.sync.dma_start(dbg_aps["dbg_kraw"][:], kraw[:])
                nc.sync.dma_start(dbg_aps["dbg_vsb0"][:], vsbs[0][:])
                nc.sync.dma_start(dbg_aps["dbg_vsb1"][:], vsbs[1][:])
                nc.sync.dma_start(
                    dbg_aps["dbg_vflat"].rearrange("(rt p) o -> p rt o", p=128),
                    vflat[:].rearrange("p (rt o) -> p rt o", rt=4))

            # ---------- phase 2: RoPE (in place: raw tiles become rot) ----------
            for raw in (qraw, kraw):
                sw = ropep.tile([128, S], bf16, tag="rope")
                nc.vector.tensor_copy(sw[0:32, :], raw[32:64, :])
                nc.vector.tensor_copy(sw[32:64, :], raw[0:32, :])
                nc.vector.tensor_copy(sw[64:96, :], raw[96:128, :])
                nc.vector.tensor_copy(sw[96:128, :], raw[64:96, :])
                nc.vector.tensor_mul(sw[:], sw[:], cs2_sb[:])
                nc.vector.tensor_mul(raw[:], raw[:], cs1_sb[:])
                nc.vector.tensor_add(raw[:], raw[:], sw[:])
            qrot, krot = qraw, kraw
            if dbg:
                nc.sync.dma_start(dbg_aps["dbg_qrot"][:], qrot[:])
                nc.sync.dma_start(dbg_aps["dbg_krot"][:], krot[:])

            # ---------- phase 3: attention ----------
            outTs = []
            with (
                tc.tile_pool(name="pssc", bufs=2, space="PSUM") as pssc,
                tc.tile_pool(name="psout", bufs=2, space="PSUM") as psout,
            ):
                for h in range(HPC):
                    outT = big.tile([64, S], bf16, tag="big")
                    outTs.append(outT)
                for b in range(8):
                    nt = 4 * (b + 1)
                    outp = [psout.tile([65, 512], f32, tag="out",
                                       name=f"outp{b}_{hh}")
                            for hh in range(HPC)]
                    slots = [(t, h) for t in range(nt) for h in range(HPC)]
                    chunks = [slots[i:i + 3] for i in range(0, len(slots), 3)]
                    for chunk in chunks:
                        W = 512 * len(chunk)
                        ps = pssc.tile([128, 1536], f32, tag="sc")
                        pch = ppool.tile([128, 1536], bf16, tag="pp")
                        for j, (t, h) in enumerate(chunk):
                            m = max(0, 128 * t - 512 * b)
                            nc.tensor.matmul(
                                ps[:, 512 * j + m: 512 * (j + 1)],
                                krot[64 * h: 64 * h + 64, 128 * t: 128 * t + 128],
                                qrot[64 * h: 64 * h + 64, 512 * b + m: 512 * (b + 1)],
                                start=True, stop=True,
                            )
                            if m or t == 4 * b:   # diagonal tile: triangle mask
                                nc.vector.tensor_add(
                                    ps[:, 512 * j + m: 512 * j + m + 128],
                                    ps[:, 512 * j + m: 512 * j + m + 128],
                                    tri_sb[:])
                        nc.scalar.activation(pch[:, 0:W], ps[:, 0:W], EXP,
                                             scale=0.125)
                        if dbg and b == 0 and chunk[0] == (0, 0):
                            stg = ppool.tile([128, 1536], f32, tag="dbgstg",
                                             name="dbgstg")
                            nc.vector.tensor_copy(stg[:], ps[:])
                            nc.sync.dma_start(dbg_aps["dbg_ps0"][:], stg[:])
                            nc.sync.dma_start(dbg_aps["dbg_pch0"][:], pch[:])
                        for j, (t, h) in enumerate(chunk):
                            m = max(0, 128 * t - 512 * b)
                            nc.tensor.matmul(
                                outp[h][:, m:512],
                                vsbs[h][:, 65 * t: 65 * t + 65],
                                pch[:, 512 * j + m: 512 * (j + 1)],
                                start=(t == 0), stop=(t == nt - 1),
                            )
                    # normalize block b
                    for h in range(HPC):
                        if dbg and b == 0 and h == 0:
                            stg2 = ppool.tile([65, 512], f32, tag="dbgstg",
                                              name="dbgstg2")
                            nc.vector.tensor_copy(stg2[:], outp[0][:])
                            nc.sync.dma_start(dbg_aps["dbg_outp0"][:], stg2[:])
                        dnr = misc[0:1, 64 + 512: 64 + 1024]  # raw denom row
                        dn = misc[0:1, 64: 64 + 512]
                        nc.vector.tensor_copy(dnr, outp[h][64:65, :])
                        nc.vector.reciprocal_approx_fast(out=dn, in_=dnr)
                        bc = pssc.tile([64, 512], f32, tag="sc")
                        nc.tensor.matmul(bc[:], ones_row, dn, start=True, stop=True)
                        if dbg and b == 0 and h == 0:
                            stg3 = ppool.tile([64, 512], f32, tag="dbgstg",
                                              name="dbgstg3")
                            nc.vector.tensor_copy(stg3[:], bc[:])
                            nc.sync.dma_start(dbg_aps["dbg_dn0"][:], dn)
                            nc.sync.dma_start(dbg_aps["dbg_bc0"][:], stg3[:])
                        osl = outTs[h][:, 512 * b: 512 * (b + 1)]
                        nc.vector.tensor_copy(osl, outp[h][0:64, :])
                        nc.vector.tensor_mul(osl, osl, bc[:])

            if dbg:
                nc.sync.dma_start(dbg_aps["dbg_outT0"][:], outTs[0][:])
                nc.sync.dma_start(dbg_aps["dbg_outT1"][:], outTs[1][:])

            # ---------- phase 4: output projection ----------
            y_sb = big.tile([128, 4 * D], f32, tag="big")     # [p, g*1024+o]
            with tc.tile_pool(name="psy", bufs=2, space="PSUM") as psy:
                for h in range(HPC):
                    ov = outTs[h][:].rearrange("p (r c) -> p r c", c=16)
                    for rt in range(2):
                        g = 2 * h + rt
                        ct = ctp.tile([128, 8 * 128], bf16, tag="ct")
                        for tp in range(8):
                            # even c -> partitions [0:64), odd -> [64:128)
                            nc.scalar.activation(
                                ct[0:64, 128 * tp: 128 * tp + 128],
                                ov[0:64, 128 * rt: 128 * rt + 128, 2 * tp], CPY)
                            nc.vector.tensor_copy(
                                ct[64:128, 128 * tp: 128 * tp + 128],
                                ov[0:64, 128 * rt: 128 * rt + 128, 2 * tp + 1])
                        for ob in range(2):
                            py = psy.tile([128, 512], f32, tag="y")
                            for tp in range(8):
                                nc.tensor.matmul(
                                    py[:],
                                    ct[:, 128 * tp: 128 * tp + 128],
                                    wo_sb[:, tp * D + ob * 512: tp * D + ob * 512 + 512],
                                    start=(tp == 0), stop=(tp == 7),
                                )
                            nc.scalar.activation(
                                y_sb[:, g * D + ob * 512: g * D + ob * 512 + 512],
                                py[:], CPY)
            nc.sync.dma_start(y.rearrange("(g p) o -> p g o", p=128),
                              y_sb[:].rearrange("p (g o) -> p g o", g=4))

    nc.compile()
    return nc


def kernel(**inputs):
    x = np.asarray(inputs["x"], dtype=np.float32)     # [1, 4096, 1024]
    Wq = np.asarray(inputs["Wq"], dtype=np.float32)
    Wk = np.asarray(inputs["Wk"], dtype=np.float32)
    Wv = np.asarray(inputs["Wv"], dtype=np.float32)
    Wo = np.asarray(inputs["Wo"], dtype=np.float32)
    # biases are structurally zero in this problem; fold anyway if nonzero
    for bn in ("bq", "bk", "bv", "bo"):
        bv_ = np.asarray(inputs.get(bn, 0.0))
        assert np.all(bv_ == 0.0), f"{bn} nonzero: unsupported"

    from concourse.bass_utils import run_bass_kernel_spmd

    if "nc" not in _CACHE:
        _CACHE["nc"] = _build_program()
    nc = _CACHE["nc"]

    bf = ml_dtypes.bfloat16
    wqT, wkT, wvT, woT, cs1, cs2, tri = _host_arrays(Wq, Wk, Wv, Wo)
    shared = {"wqT": wqT, "wkT": wkT, "wvT": wvT, "woT": woT,
              "cs1": cs1, "cs2": cs2, "tri": tri}
    xf = x.reshape(S, D)
    in_maps = []
    for i in range(NC_N):
        xTi = np.ascontiguousarray(xf[i * RPC:(i + 1) * RPC, :].T).astype(bf)
        in_maps.append(dict(shared, xT=xTi))

    trace = bool(int(os.environ.get("BASS_KERNEL_TRACE", "0")))
    res = run_bass_kernel_spmd(nc, in_maps, core_ids=list(range(NC_N)),
                               trace=trace)
    _CACHE["last_res"] = res
    if trace and res.exec_time_ns is not None:
        print(f"HW exec time: {res.exec_time_ns} ns")
        _CACHE["exec_time_ns"] = res.exec_time_ns
        _CACHE["trace"] = res.instructions_and_trace
    out = np.concatenate([res.results[i]["y"] for i in range(NC_N)], axis=0)
    return out.reshape(1, S, D).astype(np.float32)


# revision 23
# speedup vs baseline: 1.1207x; 1.1207x over previous
"""Causal self-attention (quirky-reshape variant) on 8 TRN2 NeuronCores.

Key structural fact: the reference reshapes (B,S,H*dk) -> (B,H,S,dk) without a
transpose, so head h's Q/K/V come from rows [256h, 256h+256) of the [4096,1024]
projection output (reinterpreted as [4096,64]), and output rows [256h, 256h+256)
depend only on head h.  With 2 heads per core the problem is embarrassingly
parallel: core i consumes x rows [512i, 512i+512) + full weights and produces
output rows [512i, 512i+512).  No collectives.

Per-core pipeline (all matmuls in the "transposed-scores" orientation):
  qflat^T/kflat^T via o-stationary projection (host-permuted weights fold both
  the quirky reshape's d-extraction and a RoPE even/odd de-interleave into the
  PSUM partition order), vflat via r-stationary projection; RoPE as 3 full-width
  DVE passes; scores^T = K^T-stationary matmul (2 heads row-packed in the PE
  array); exp on ScalarE (scale=1/8 folded in, causal triangle masks added on
  PSUM, fully-masked columns skipped by ragged matmul widths); P@V with a
  [V|ones] stationary (ones column yields softmax denominators); normalize with
  reciprocal_approx_fast + a PE outer-product broadcast; output projection from
  strided-gathered concat^T tiles.
"""

import os

os.environ.setdefault("JAX_PLATFORMS", "cpu")

import numpy as np
import ml_dtypes

D = 1024          # d_model
H = 16            # heads
DK = 64           # head dim
S = 4096          # seq len
NC_N = 8          # cores
RPC = 512         # x rows per core
HPC = 2           # heads per core
NT_SK = 32        # sk tiles of 128 per head
ROPE_THETA = 10000.0
F32 = None        # set at build (mybir.dt.float32)
BF16 = None

_CACHE = {}


def _deint_perm():
    """o' -> o source index: within each 64-block, evens first then odds."""
    d_order = list(range(0, DK, 2)) + list(range(1, DK, 2))  # position d' -> d
    perm = np.zeros(D, dtype=np.int64)
    for c in range(H):
        for dp, d in enumerate(d_order):
            perm[c * DK + dp] = c * DK + d
    return perm


def _host_arrays(Wq, Wk, Wv, Wo):
    bf = ml_dtypes.bfloat16
    perm = _deint_perm()
    wqT = np.ascontiguousarray(Wq[perm, :].T).astype(bf)   # [in, o'] deint
    wkT = np.ascontiguousarray(Wk[perm, :].T).astype(bf)
    wvT = np.ascontiguousarray(Wv.T).astype(bf)            # [in, o] natural
    woT = np.ascontiguousarray(Wo.T).astype(bf)            # [o_c, o_out]

    # RoPE tables in the [Aev, Aod, Bev, Bod] partition grouping (32 rows each;
    # identical for both heads since the angle depends only on s).
    j = np.arange(0, DK, 2, dtype=np.float64) / DK
    inv_freq = 1.0 / (ROPE_THETA ** j)                     # [32]
    s = np.arange(S, dtype=np.float64)
    ang = np.outer(inv_freq, s)                            # [32, S]
    cos = np.cos(ang)
    sin = np.sin(ang)
    cs1 = np.concatenate([cos, cos, cos, cos], 0).astype(bf)       # mult RAW
    cs2 = np.concatenate([-sin, sin, -sin, sin], 0).astype(bf)     # mult SWAP
    # triangle mask: scores^T[p, q] valid iff p <= q (within a diagonal block)
    tri = np.where(
        np.arange(128)[:, None] <= np.arange(128)[None, :], 0.0, -1e30
    ).astype(np.float32)
    return wqT, wkT, wvT, woT, cs1, cs2, tri


def _build_program(dbg=False):
    import concourse.bass as bass
    import concourse.tile as tile
    from concourse import bacc, mybir

    f32 = mybir.dt.float32
    bf16 = mybir.dt.bfloat16
    EXP = mybir.ActivationFunctionType.Exp
    CPY = mybir.ActivationFunctionType.Copy

    nc = bacc.Bacc("TRN2", target_bir_lowering=False, debug=False,
                   num_devices=NC_N)

    xT = nc.dram_tensor("xT", [D, RPC], bf16, kind="ExternalInput").ap()
    wq = nc.dram_tensor("wqT", [D, D], bf16, kind="ExternalInput").ap()
    wk = nc.dram_tensor("wkT", [D, D], bf16, kind="ExternalInput").ap()
    wv = nc.dram_tensor("wvT", [D, D], bf16, kind="ExternalInput").ap()
    wo = nc.dram_tensor("woT", [D, D], bf16, kind="ExternalInput").ap()
    cs1d = nc.dram_tensor("cs1", [128, S], bf16, kind="ExternalInput").ap()
    cs2d = nc.dram_tensor("cs2", [128, S], bf16, kind="ExternalInput").ap()
    trid = nc.dram_tensor("tri", [128, 128], f32, kind="ExternalInput").ap()
    y = nc.dram_tensor("y", [RPC, D], f32, kind="ExternalOutput").ap()
    vfd = nc.dram_tensor("vflat_scratch", [RPC, D], bf16).ap()
    dbg_aps = {}
    if dbg:
        for nm, shp, dt in [
            ("dbg_qraw", [128, S], bf16), ("dbg_kraw", [128, S], bf16),
            ("dbg_qrot", [128, S], bf16), ("dbg_krot", [128, S], bf16),
            ("dbg_vsb0", [128, 65 * NT_SK], bf16),
            ("dbg_vsb1", [128, 65 * NT_SK], bf16),
            ("dbg_outT0", [64, S], bf16), ("dbg_outT1", [64, S], bf16),
            ("dbg_vflat", [RPC, D], bf16),
            ("dbg_outp0", [65, 512], f32), ("dbg_ps0", [128, 1536], f32),
            ("dbg_pch0", [128, 1536], bf16), ("dbg_dn0", [1, 512], f32),
            ("dbg_bc0", [64, 512], f32),
        ]:
            dbg_aps[nm] = nc.dram_tensor(nm, shp, dt, kind="ExternalOutput").ap()

    with tile.TileContext(nc) as tc:
        with (
            tc.tile_pool(name="big", bufs=3) as big,        # wq/wk/wv -> outTA/outTB/y_sb
            tc.tile_pool(name="wo", bufs=1) as wop,
            tc.tile_pool(name="xp", bufs=1) as xp,
            tc.tile_pool(name="qk", bufs=2) as qkp,          # qraw, kraw (become rot in place)
            tc.tile_pool(name="rope", bufs=4) as ropep,      # cs1, cs2, swQ, swK
            tc.tile_pool(name="vf", bufs=1) as vfp,
            tc.tile_pool(name="vsb", bufs=2) as vsbp,
            tc.tile_pool(name="mask", bufs=1) as maskp,
            tc.tile_pool(name="pp", bufs=3) as ppool,        # exp'd P chunks
            tc.tile_pool(name="ct", bufs=2) as ctp,          # concatT per (h, rt)
            tc.tile_pool(name="misc", bufs=1) as miscp,
        ):
            # ---------- phase 0: loads ----------
            xsb = xp.tile([128, 8 * RPC], bf16, tag="x")           # [p, kt*512+r]
            nc.sync.dma_start(xsb[:].rearrange("p (kt r) -> p kt r", kt=8),
                              xT.rearrange("(kt p) r -> p kt r", p=128))

            def load_w(pool, tag, src):
                t = pool.tile([128, 8 * D], bf16, tag=tag, name=f"w_{tag}")
                tv = t[:].rearrange("p (kt o) -> p kt o", kt=8)
                sv = src.rearrange("(kt p) o -> p kt o", p=128)
                for kt in range(8):
                    nc.sync.dma_start(tv[:, kt, :], sv[:, kt, :])
                return t

            wq_sb = load_w(big, "big", wq)
            wk_sb = load_w(big, "big", wk)
            wv_sb = load_w(big, "big", wv)
            wo_sb = load_w(wop, "wo", wo)
            cs1_sb = ropep.tile([128, S], bf16, tag="rope")
            nc.sync.dma_start(cs1_sb[:], cs1d[:])
            cs2_sb = ropep.tile([128, S], bf16, tag="rope")
            nc.sync.dma_start(cs2_sb[:], cs2d[:])
            tri_sb = maskp.tile([128, 128], f32, tag="mask")
            nc.sync.dma_start(tri_sb[:], trid[:])

            misc = miscp.tile([128, 2048], f32, tag="misc")
            nc.gpsimd.memset(misc[:], 0.0)
            bc_sb = miscp.tile([64, 512], f32, tag="bc")

            # ---------- phase 1: projections ----------
            qraw = qkp.tile([128, S], bf16, tag="qk")   # [Aev,Aod,Bev,Bod] x s
            kraw = qkp.tile([128, S], bf16, tag="qk")

            with tc.tile_pool(name="psproj", bufs=3, space="PSUM") as psp:
                for w_sb, raw in ((wq_sb, qraw), (wk_sb, kraw)):
                    rv = raw[:].rearrange("p (r c) -> p r c", c=16)
                    for ot in range(8):
                        pq = psp.tile([128, RPC], f32, tag="ps")
                        for kt in range(8):
                            nc.tensor.matmul(
                                pq[:],
                                w_sb[:, kt * D + ot * 128: kt * D + ot * 128 + 128],
                                xsb[:, kt * RPC: (kt + 1) * RPC],
                                start=(kt == 0), stop=(kt == 7),
                            )
                        c0 = 2 * ot
                        # (A, c0): no partition shift -> ScalarE
                        nc.scalar.activation(rv[0:64, 0:256, c0], pq[0:64, 0:256], CPY)
                        # (B, c0): shift 0->64 -> DVE
                        nc.vector.tensor_copy(rv[64:128, 0:256, c0], pq[0:64, 256:512])
                        # (A, c0+1): shift 64->0 -> DVE
                        nc.vector.tensor_copy(rv[0:64, 0:256, c0 + 1], pq[64:128, 0:256])
                        # (B, c0+1): no shift -> ScalarE
                        nc.scalar.activation(rv[64:128, 0:256, c0 + 1], pq[64:128, 256:512], CPY)

                # V projection, r-stationary: vflat [r, o] natural
                vflat = vfp.tile([128, 4 * D], bf16, tag="vf")    # [p, rt*1024+o]
                for rt in range(4):
                    for ob in range(2):
                        pv = psp.tile([128, 512], f32, tag="ps")
                        for kt in range(8):
                            nc.tensor.matmul(
                                pv[:],
                                xsb[:, kt * RPC + rt * 128: kt * RPC + rt * 128 + 128],
                                wv_sb[:, kt * D + ob * 512: kt * D + ob * 512 + 512],
                                start=(kt == 0), stop=(kt == 7),
                            )
                        nc.scalar.activation(
                            vflat[:, rt * D + ob * 512: rt * D + ob * 512 + 512],
                            pv[:], CPY)

            # V reshape through DRAM: vflat [r,o] -> vsb_h[p, 65T+d] ([V|ones])
            nc.sync.dma_start(vfd.rearrange("(rt p) o -> p rt o", p=128),
                              vflat[:].rearrange("p (rt o) -> p rt o", rt=4))
            vsbs = []
            vld = vfd.rearrange("(h T a) (c d) -> h a c T d", h=2, T=32, a=8,
                                c=16, d=DK)
            for h in range(HPC):
                vsb = vsbp.tile([128, 65 * NT_SK], bf16, tag="vsb")
                nc.gpsimd.memset(vsb[:], 1.0)   # ones col at 65T+64 survives
                dstv = vsb[:].rearrange("(a c) (T d) -> a c T d", a=8, c=16,
                                        T=NT_SK, d=65)
                for a in range(8):
                    nc.sync.dma_start(dstv[a, :, :, 0:DK], vld[h, a])
                vsbs.append(vsb)

            if dbg:
                nc.sync.dma_start(dbg_aps["dbg_qraw"][:], qraw[:])
                nc.sync.dma_start(dbg_aps["dbg_kraw"][:], kraw[:])
                nc.sync.dma_start(dbg_aps["dbg_vsb0"][:], vsbs[0][:])
                nc.sync.dma_start(dbg_aps["dbg_vsb1"][:], vsbs[1][:])
                nc.sync.dma_start(
                    dbg_aps["dbg_vflat"].rearrange("(rt p) o -> p rt o", p=128),
                    vflat[:].rearrange("p (rt o) -> p rt o", rt=4))

            # ---------- phase 2: RoPE (in place: raw tiles become rot) ----------
            for raw in (qraw, kraw):
                sw = ropep.tile([128, S], bf16, tag="rope")
                nc.vector.tensor_copy(sw[0:32, :], raw[32:64, :])
                nc.vector.tensor_copy(sw[32:64, :], raw[0:32, :])
                nc.vector.tensor_copy(sw[64:96, :], raw[96:128, :])
                nc.vector.tensor_copy(sw[96:128, :], raw[64:96, :])
                nc.vector.tensor_mul(sw[:], sw[:], cs2_sb[:])
                nc.vector.tensor_mul(raw[:], raw[:], cs1_sb[:])
                nc.vector.tensor_add(raw[:], raw[:], sw[:])
            qrot, krot = qraw, kraw
            if dbg:
                nc.sync.dma_start(dbg_aps["dbg_qrot"][:], qrot[:])
                nc.sync.dma_start(dbg_aps["dbg_krot"][:], krot[:])

            # ---------- phase 3: attention ----------
            outTs = []
            with (
                tc.tile_pool(name="pssc", bufs=2, space="PSUM") as pssc,
                tc.tile_pool(name="psout", bufs=2, space="PSUM") as psout,
            ):
                for h in range(HPC):
                    outT = big.tile([64, S], bf16, tag="big")
                    outTs.append(outT)
                for b in range(8):
                    nt = 4 * (b + 1)
                    outp = [psout.tile([65, 512], f32, tag="out",
                                       name=f"outp{b}_{hh}")
                            for hh in range(HPC)]
                    slots = [(t, h) for t in range(nt) for h in range(HPC)]
                    chunks = [slots[i:i + 3] for i in range(0, len(slots), 3)]
                    for chunk in chunks:
                        W = 512 * len(chunk)
                        ps = pssc.tile([128, 1536], f32, tag="sc")
                        pch = ppool.tile([128, 1536], bf16, tag="pp")
                        for j, (t, h) in enumerate(chunk):
                            m = max(0, 128 * t - 512 * b)
                            nc.tensor.matmul(
                                ps[:, 512 * j + m: 512 * (j + 1)],
                                krot[64 * h: 64 * h + 64, 128 * t: 128 * t + 128],
                                qrot[64 * h: 64 * h + 64, 512 * b + m: 512 * (b + 1)],
                                start=True, stop=True,
                            )
                            if m or t == 4 * b:   # diagonal tile: triangle mask
                                nc.vector.tensor_add(
                                    ps[:, 512 * j + m: 512 * j + m + 128],
                                    ps[:, 512 * j + m: 512 * j + m + 128],
                                    tri_sb[:])
                        nc.scalar.activation(pch[:, 0:W], ps[:, 0:W], EXP,
                                             scale=0.125)
                        if dbg and b == 0 and chunk[0] == (0, 0):
                            stg = ppool.tile([128, 1536], f32, tag="dbgstg",
                                             name="dbgstg")
                            nc.vector.tensor_copy(stg[:], ps[:])
                            nc.sync.dma_start(dbg_aps["dbg_ps0"][:], stg[:])
                            nc.sync.dma_start(dbg_aps["dbg_pch0"][:], pch[:])
                        for j, (t, h) in enumerate(chunk):
                            m = max(0, 128 * t - 512 * b)
                            nc.tensor.matmul(
                                outp[h][:, m:512],
                                vsbs[h][:, 65 * t: 65 * t + 65],
                                pch[:, 512 * j + m: 512 * (j + 1)],
                                start=(t == 0), stop=(t == nt - 1),
                            )
                    # normalize block b
                    for h in range(HPC):
                        if dbg and b == 0 and h == 0:
                            stg2 = ppool.tile([65, 512], f32, tag="dbgstg",
                                              name="dbgstg2")
                            nc.vector.tensor_copy(stg2[:], outp[0][:])
                            nc.sync.dma_start(dbg_aps["dbg_outp0"][:], stg2[:])
                        dnr = misc[0:1, 64 + 512: 64 + 1024]  # raw denom row
                        dn = misc[0:1, 64: 64 + 512]
                        nc.vector.tensor_copy(dnr, outp[h][64:65, :])
                        nc.vector.reciprocal_approx_fast(out=dn, in_=dnr)
                        # broadcast dn across 64 partitions: lane-0 shuffle x2
                        nc.vector.stream_shuffle(bc_sb[0:32, :],
                                                 misc[0:32, 64:576], [0] * 32)
                        nc.vector.stream_shuffle(bc_sb[32:64, :],
                                                 misc[0:32, 64:576], [0] * 32)
                        if dbg and b == 0 and h == 0:
                            nc.sync.dma_start(dbg_aps["dbg_dn0"][:], dn)
                            nc.sync.dma_start(dbg_aps["dbg_bc0"][:], bc_sb[:])
                        osl = outTs[h][:, 512 * b: 512 * (b + 1)]
                        nc.vector.tensor_mul(osl, outp[h][0:64, :], bc_sb[:])

            if dbg:
                nc.sync.dma_start(dbg_aps["dbg_outT0"][:], outTs[0][:])
                nc.sync.dma_start(dbg_aps["dbg_outT1"][:], outTs[1][:])

            # ---------- phase 4: output projection ----------
            y_sb = big.tile([128, 4 * D], f32, tag="big")     # [p, g*1024+o]
            with tc.tile_pool(name="psy", bufs=2, space="PSUM") as psy:
                for h in range(HPC):
                    ov = outTs[h][:].rearrange("p (r c) -> p r c", c=16)
                    for rt in range(2):
                        g = 2 * h + rt
                        ct = ctp.tile([128, 8 * 128], bf16, tag="ct")
                        for tp in range(8):
                            # even c -> partitions [0:64), odd -> [64:128)
                            nc.scalar.activation(
                                ct[0:64, 128 * tp: 128 * tp + 128],
                                ov[0:64, 128 * rt: 128 * rt + 128, 2 * tp], CPY)
                            nc.vector.tensor_copy(
                                ct[64:128, 128 * tp: 128 * tp + 128],
                                ov[0:64, 128 * rt: 128 * rt + 128, 2 * tp + 1])
                        for ob in range(2):
                            py = psy.tile([128, 512], f32, tag="y")
                            for tp in range(8):
                                nc.tensor.matmul(
                                    py[:],
                                    ct[:, 128 * tp: 128 * tp + 128],
                                    wo_sb[:, tp * D + ob * 512: tp * D + ob * 512 + 512],
                                    start=(tp == 0), stop=(tp == 7),
                                )
                            nc.scalar.activation(
                                y_sb[:, g * D + ob * 512: g * D + ob * 512 + 512],
                                py[:], CPY)
            nc.sync.dma_start(y.rearrange("(g p) o -> p g o", p=128),
                              y_sb[:].rearrange("p (g o) -> p g o", g=4))

    nc.compile()
    return nc


def kernel(**inputs):
    x = np.asarray(inputs["x"], dtype=np.float32)     # [1, 4096, 1024]
    Wq = np.asarray(inputs["Wq"], dtype=np.float32)
    Wk = np.asarray(inputs["Wk"], dtype=np.float32)
    Wv = np.asarray(inputs["Wv"], dtype=np.float32)
    Wo = np.asarray(inputs["Wo"], dtype=np.float32)
    # biases are structurally zero in this problem; fold anyway if nonzero
    for bn in ("bq", "bk", "bv", "bo"):
        bv_ = np.asarray(inputs.get(bn, 0.0))
        assert np.all(bv_ == 0.0), f"{bn} nonzero: unsupported"

    from concourse.bass_utils import run_bass_kernel_spmd

    if "nc" not in _CACHE:
        _CACHE["nc"] = _build_program()
    nc = _CACHE["nc"]

    bf = ml_dtypes.bfloat16
    wqT, wkT, wvT, woT, cs1, cs2, tri = _host_arrays(Wq, Wk, Wv, Wo)
    shared = {"wqT": wqT, "wkT": wkT, "wvT": wvT, "woT": woT,
              "cs1": cs1, "cs2": cs2, "tri": tri}
    xf = x.reshape(S, D)
    in_maps = []
    for i in range(NC_N):
        xTi = np.ascontiguousarray(xf[i * RPC:(i + 1) * RPC, :].T).astype(bf)
        in_maps.append(dict(shared, xT=xTi))

    trace = bool(int(os.environ.get("BASS_KERNEL_TRACE", "0")))
    res = run_bass_kernel_spmd(nc, in_maps, core_ids=list(range(NC_N)),
                               trace=trace)
    _CACHE["last_res"] = res
    if trace and res.exec_time_ns is not None:
        print(f"HW exec time: {res.exec_time_ns} ns")
        _CACHE["exec_time_ns"] = res.exec_time_ns
        _CACHE["trace"] = res.instructions_and_trace
    out = np.concatenate([res.results[i]["y"] for i in range(NC_N)], axis=0)
    return out.reshape(1, S, D).astype(np.float32)


# revision 43
# speedup vs baseline: 1.1962x; 1.0674x over previous
"""Causal self-attention (quirky-reshape variant) on 8 TRN2 NeuronCores.

Key structural fact: the reference reshapes (B,S,H*dk) -> (B,H,S,dk) without a
transpose, so head h's Q/K/V come from rows [256h, 256h+256) of the [4096,1024]
projection output (reinterpreted as [4096,64]), and output rows [256h, 256h+256)
depend only on head h.  With 2 heads per core the problem is embarrassingly
parallel: core i consumes x rows [512i, 512i+512) + full weights and produces
output rows [512i, 512i+512).  No collectives.

Per-core pipeline (all matmuls in the "transposed-scores" orientation):
  qflat^T/kflat^T via o-stationary projection (host-permuted weights fold both
  the quirky reshape's d-extraction and a RoPE even/odd de-interleave into the
  PSUM partition order), vflat via r-stationary projection; RoPE as 3 full-width
  DVE passes; scores^T = K^T-stationary matmul (2 heads row-packed in the PE
  array); exp on ScalarE (scale=1/8 folded in, causal triangle masks added on
  PSUM, fully-masked columns skipped by ragged matmul widths); P@V with a
  [V|ones] stationary (ones column yields softmax denominators); normalize with
  reciprocal_approx_fast + a PE outer-product broadcast; output projection from
  strided-gathered concat^T tiles.
"""

import os

os.environ.setdefault("JAX_PLATFORMS", "cpu")

import numpy as np
import ml_dtypes

D = 1024          # d_model
H = 16            # heads
DK = 64           # head dim
S = 4096          # seq len
NC_N = 8          # cores
RPC = 512         # x rows per core
HPC = 2           # heads per core
NT_SK = 32        # sk tiles of 128 per head
ROPE_THETA = 10000.0
F32 = None        # set at build (mybir.dt.float32)
BF16 = None

_CACHE = {}


def _deint_perm():
    """o' -> o source index: within each 64-block, evens first then odds."""
    d_order = list(range(0, DK, 2)) + list(range(1, DK, 2))  # position d' -> d
    perm = np.zeros(D, dtype=np.int64)
    for c in range(H):
        for dp, d in enumerate(d_order):
            perm[c * DK + dp] = c * DK + d
    return perm


def _host_arrays(Wq, Wk, Wv, Wo):
    bf = ml_dtypes.bfloat16
    perm = _deint_perm()
    wqT = np.ascontiguousarray(Wq[perm, :].T).astype(bf)   # [in, o'] deint
    wkT = np.ascontiguousarray(Wk[perm, :].T).astype(bf)
    wvT = np.ascontiguousarray(Wv.T).astype(bf)            # [in, o] natural
    woT = np.ascontiguousarray(Wo.T).astype(bf)            # [o_c, o_out]

    # RoPE tables in the [Aev, Aod, Bev, Bod] partition grouping (32 rows each;
    # identical for both heads since the angle depends only on s).
    j = np.arange(0, DK, 2, dtype=np.float64) / DK
    inv_freq = 1.0 / (ROPE_THETA ** j)                     # [32]
    # Q storage is c-major: u = 256*c + r <-> s = 16*r + c; K is s-ordered
    u = np.arange(S)
    s_of_u = 16 * (u % 256) + u // 256
    angq = np.outer(inv_freq, s_of_u)                      # [32, S] u-ordered
    angk = np.outer(inv_freq, np.arange(S))                # [32, S] s-ordered
    csq1 = np.concatenate([np.cos(angq)] * 4, 0).astype(bf)
    csq2 = np.concatenate([-np.sin(angq), np.sin(angq)] * 2, 0).astype(bf)
    csk1 = np.concatenate([np.cos(angk)] * 4, 0).astype(bf)
    csk2 = np.concatenate([-np.sin(angk), np.sin(angk)] * 2, 0).astype(bf)
    # triangle mask: row p = local sk (plain), col j = 8*cq + rq (c-major sq)
    p = np.arange(128)
    cq, rq = np.arange(128) // 8, np.arange(128) % 8
    sq_loc = 16 * rq + cq                                  # [128]
    tri = np.where(
        p[:, None] <= sq_loc[None, :], 0.0, -1e30
    ).astype(np.float32)
    return wqT, wkT, wvT, woT, csq1, csq2, csk1, csk2, tri


def _build_program(dbg=False):
    import concourse.bass as bass
    import concourse.tile as tile
    from concourse import bacc, mybir

    f32 = mybir.dt.float32
    bf16 = mybir.dt.bfloat16
    EXP = mybir.ActivationFunctionType.Exp
    CPY = mybir.ActivationFunctionType.Copy

    nc = bacc.Bacc("TRN2", target_bir_lowering=False, debug=False,
                   num_devices=NC_N)

    xT = nc.dram_tensor("xT", [D, RPC], bf16, kind="ExternalInput").ap()
    wq = nc.dram_tensor("wqT", [D, D], bf16, kind="ExternalInput").ap()
    wk = nc.dram_tensor("wkT", [D, D], bf16, kind="ExternalInput").ap()
    wv = nc.dram_tensor("wvT", [D, D], bf16, kind="ExternalInput").ap()
    wo = nc.dram_tensor("woT", [D, D], bf16, kind="ExternalInput").ap()
    cs1d = nc.dram_tensor("cs1", [128, S], bf16, kind="ExternalInput").ap()
    cs2d = nc.dram_tensor("cs2", [128, S], bf16, kind="ExternalInput").ap()
    cs3d = nc.dram_tensor("cs3", [128, S], bf16, kind="ExternalInput").ap()
    cs4d = nc.dram_tensor("cs4", [128, S], bf16, kind="ExternalInput").ap()
    trid = nc.dram_tensor("tri", [128, 128], f32, kind="ExternalInput").ap()
    y = nc.dram_tensor("y", [RPC, D], f32, kind="ExternalOutput").ap()
    vfd = nc.dram_tensor("vflat_scratch", [RPC, D], bf16).ap()
    dbg_aps = {}
    if dbg:
        for nm, shp, dt in [
            ("dbg_qraw", [128, S], bf16), ("dbg_kraw", [128, S], bf16),
            ("dbg_qrot", [128, S], bf16), ("dbg_krot", [128, S], bf16),
            ("dbg_vsb0", [128, 65 * NT_SK], bf16),
            ("dbg_vsb1", [128, 65 * NT_SK], bf16),
            ("dbg_outT0", [64, S], bf16), ("dbg_outT1", [64, S], bf16),
            ("dbg_vflat", [RPC, D], bf16),
            ("dbg_outp0", [65, 512], f32), ("dbg_ps0", [128, 1536], f32),
            ("dbg_pch0", [128, 1536], bf16), ("dbg_dn0", [1, 512], f32),
            ("dbg_bc0", [64, 512], f32),
        ]:
            dbg_aps[nm] = nc.dram_tensor(nm, shp, dt, kind="ExternalOutput").ap()

    with tile.TileContext(nc) as tc:
        with (
            tc.tile_pool(name="big", bufs=3) as big,        # wq/wk/wv -> outTA/outTB/y_sb
            tc.tile_pool(name="wo", bufs=1) as wop,
            tc.tile_pool(name="xp", bufs=1) as xp,
            tc.tile_pool(name="qk", bufs=2) as qkp,          # qraw, kraw (become rot in place)
            tc.tile_pool(name="rope", bufs=5) as ropep,      # cs tables + swap
            tc.tile_pool(name="vf", bufs=1) as vfp,
            tc.tile_pool(name="vsb", bufs=2) as vsbp,
            tc.tile_pool(name="mask", bufs=1) as maskp,
            tc.tile_pool(name="pp", bufs=3) as ppool,        # exp'd P chunks
            tc.tile_pool(name="ct", bufs=2) as ctp,          # concatT per (h, rt)
            tc.tile_pool(name="misc", bufs=1) as miscp,
        ):
            # ---------- phase 0: loads ----------
            xsb = xp.tile([128, 8 * RPC], bf16, tag="x")           # [p, kt*512+r]
            nc.sync.dma_start(xsb[:].rearrange("p (kt r) -> p kt r", kt=8),
                              xT.rearrange("(kt p) r -> p kt r", p=128))

            def load_w(pool, tag, src):
                t = pool.tile([128, 8 * D], bf16, tag=tag, name=f"w_{tag}")
                tv = t[:].rearrange("p (kt o) -> p kt o", kt=8)
                sv = src.rearrange("(kt p) o -> p kt o", p=128)
                for kt in range(8):
                    nc.sync.dma_start(tv[:, kt, :], sv[:, kt, :])
                return t

            wq_sb = load_w(big, "big", wq)
            wk_sb = load_w(big, "big", wk)
            wv_sb = load_w(big, "big", wv)
            wo_sb = load_w(wop, "wo", wo)
            cs1_sb = ropep.tile([128, S], bf16, tag="rope")
            nc.sync.dma_start(cs1_sb[:], cs1d[:])
            cs2_sb = ropep.tile([128, S], bf16, tag="rope")
            nc.sync.dma_start(cs2_sb[:], cs2d[:])
            cs3_sb = ropep.tile([128, S], bf16, tag="rope")
            nc.sync.dma_start(cs3_sb[:], cs3d[:])
            cs4_sb = ropep.tile([128, S], bf16, tag="rope")
            nc.sync.dma_start(cs4_sb[:], cs4d[:])
            tri_sb = maskp.tile([128, 128], f32, tag="mask")
            nc.sync.dma_start(tri_sb[:], trid[:])

            misc = miscp.tile([128, 2048], f32, tag="misc")
            nc.gpsimd.memset(misc[:], 0.0)
            bc_sb = miscp.tile([64, 512], f32, tag="bc")

            # ---------- phase 1: projections ----------
            qraw = qkp.tile([128, S], bf16, tag="qk")   # [Aev,Aod,Bev,Bod] x s
            kraw = qkp.tile([128, S], bf16, tag="qk")

            with tc.tile_pool(name="psproj", bufs=3, space="PSUM") as psp:
                for w_sb, raw, cmajor in ((wq_sb, qraw, True),
                                          (wk_sb, kraw, False)):
                    kv_raw = raw[:].rearrange("p (r c) -> p r c", c=16)
                    for ot in range(8):
                        pq = psp.tile([128, RPC], f32, tag="ps")
                        for kt in range(8):
                            nc.tensor.matmul(
                                pq[:],
                                w_sb[:, kt * D + ot * 128: kt * D + ot * 128 + 128],
                                xsb[:, kt * RPC: (kt + 1) * RPC],
                                start=(kt == 0), stop=(kt == 7),
                            )
                        c0 = 2 * ot
                        if cmajor:   # Q: u = 256*c + r, contiguous copies
                            u0, u1 = 256 * c0, 256 * (c0 + 1)
                            nc.scalar.activation(raw[0:64, u0:u0 + 256],
                                                 pq[0:64, 0:256], CPY)
                            nc.vector.tensor_copy(raw[64:128, u0:u0 + 256],
                                                  pq[0:64, 256:512])
                            nc.vector.tensor_copy(raw[0:64, u1:u1 + 256],
                                                  pq[64:128, 0:256])
                            nc.scalar.activation(raw[64:128, u1:u1 + 256],
                                                 pq[64:128, 256:512], CPY)
                        else:        # K: s-ordered, strided dst (step 16)
                            nc.scalar.activation(kv_raw[0:64, 0:256, c0],
                                                 pq[0:64, 0:256], CPY)
                            nc.vector.tensor_copy(kv_raw[64:128, 0:256, c0],
                                                  pq[0:64, 256:512])
                            nc.vector.tensor_copy(kv_raw[0:64, 0:256, c0 + 1],
                                                  pq[64:128, 0:256])
                            nc.scalar.activation(kv_raw[64:128, 0:256, c0 + 1],
                                                 pq[64:128, 256:512], CPY)

                # V projection, r-stationary: vflat [r, o] natural
                vflat = vfp.tile([128, 4 * D], bf16, tag="vf")    # [p, rt*1024+o]
                for rt in range(4):
                    for ob in range(2):
                        pv = psp.tile([128, 512], f32, tag="ps")
                        for kt in range(8):
                            nc.tensor.matmul(
                                pv[:],
                                xsb[:, kt * RPC + rt * 128: kt * RPC + rt * 128 + 128],
                                wv_sb[:, kt * D + ob * 512: kt * D + ob * 512 + 512],
                                start=(kt == 0), stop=(kt == 7),
                            )
                        nc.scalar.activation(
                            vflat[:, rt * D + ob * 512: rt * D + ob * 512 + 512],
                            pv[:], CPY)

            # V reshape through DRAM: vflat [r,o] -> vsb_h[p, 65T+d] ([V|ones])
            nc.sync.dma_start(vfd.rearrange("(rt p) o -> p rt o", p=128),
                              vflat[:].rearrange("p (rt o) -> p rt o", rt=4))
            # vsb row p = plain local sk: V row s = 128*T + p
            vsbs = []
            vld = vfd.rearrange("(h T a) (c d) -> h a c T d", h=2, T=NT_SK,
                                a=8, c=16, d=DK)
            for h in range(HPC):
                vsb = vsbp.tile([128, 65 * NT_SK], bf16, tag="vsb")
                nc.gpsimd.memset(vsb[:], 1.0)   # ones col at 65T+64 survives
                dstv = vsb[:].rearrange("(a c) (T d) -> a c T d", a=8, c=16,
                                        T=NT_SK, d=65)
                for a in range(8):
                    nc.sync.dma_start(dstv[a, :, :, 0:DK], vld[h, a])
                vsbs.append(vsb)

            if dbg:
                nc.sync.dma_start(dbg_aps["dbg_qraw"][:], qraw[:])
                nc.sync.dma_start(dbg_aps["dbg_kraw"][:], kraw[:])
                nc.sync.dma_start(dbg_aps["dbg_vsb0"][:], vsbs[0][:])
                nc.sync.dma_start(dbg_aps["dbg_vsb1"][:], vsbs[1][:])
                nc.sync.dma_start(
                    dbg_aps["dbg_vflat"].rearrange("(rt p) o -> p rt o", p=128),
                    vflat[:].rearrange("p (rt o) -> p rt o", rt=4))

            # ---------- phase 2: RoPE (in place: raw tiles become rot) ----------
            for raw, c1, c2 in ((qraw, cs1_sb, cs2_sb), (kraw, cs3_sb, cs4_sb)):
                sw = ropep.tile([128, S], bf16, tag="rope")
                nc.vector.tensor_copy(sw[0:32, :], raw[32:64, :])
                nc.vector.tensor_copy(sw[32:64, :], raw[0:32, :])
                nc.vector.tensor_copy(sw[64:96, :], raw[96:128, :])
                nc.vector.tensor_copy(sw[96:128, :], raw[64:96, :])
                nc.vector.tensor_mul(sw[:], sw[:], c2[:])
                nc.vector.tensor_mul(raw[:], raw[:], c1[:])
                nc.vector.tensor_add(raw[:], raw[:], sw[:])
            qrot, krot = qraw, kraw
            if dbg:
                nc.sync.dma_start(dbg_aps["dbg_qrot"][:], qrot[:])
                nc.sync.dma_start(dbg_aps["dbg_krot"][:], krot[:])

            # ---------- phase 3: attention ----------
            outTs = []
            with (
                tc.tile_pool(name="pssc", bufs=2, space="PSUM") as pssc,
                tc.tile_pool(name="psout", bufs=2, space="PSUM") as psout,
            ):
                for h in range(HPC):
                    outT = big.tile([64, S], bf16, tag="big")
                    outTs.append(outT)
                qv = [qrot[64 * h: 64 * h + 64, :].rearrange(
                    "p (c r) -> p c r", c=16) for h in range(HPC)]
                for b in range(8):
                    nt = 4 * (b + 1)
                    outp = [psout.tile([65, 512], f32, tag="out",
                                       name=f"outp{b}_{hh}")
                            for hh in range(HPC)]
                    opv = [outp[hh][:].rearrange("p (c r) -> p c r", r=32)
                           for hh in range(HPC)]
                    slots = [(t, h) for t in range(nt) for h in range(HPC)]
                    chunks = [slots[i:i + 3] for i in range(0, len(slots), 3)]
                    for chunk in chunks:
                        ps = pssc.tile([128, 1536], f32, tag="sc")
                        pch = ppool.tile([128, 1536], bf16, tag="pp")
                        # slot layout: uniform c-major-32 groups; valid r-range
                        # [rmin, 32) per c-group, garbage at [0, rmin) (unread)
                        for j, (t, h) in enumerate(chunk):
                            rmin = 8 * max(0, t - 4 * b)   # first valid r
                            psv = ps[:, 512 * j: 512 * (j + 1)].rearrange(
                                "p (c r) -> p c r", r=32)
                            nc.tensor.matmul(
                                psv[:, :, rmin:32],
                                krot[64 * h: 64 * h + 64,
                                     128 * t: 128 * t + 128],
                                qv[h][:, :, 32 * b + rmin: 32 * (b + 1)],
                                start=True, stop=True,
                            )
                            if t >= 4 * b:   # diagonal tile: triangle mask
                                trv = psv[:, :, rmin:rmin + 8]
                                nc.vector.tensor_add(trv, trv, tri_sb[:])
                        Wtot = 512 * len(chunk)
                        nc.scalar.activation(pch[:, 0:Wtot], ps[:, 0:Wtot],
                                             EXP, scale=0.125)
                        if dbg and b == 0 and chunk[0] == (0, 0):
                            stg = ppool.tile([128, 1536], f32, tag="dbgstg",
                                             name="dbgstg")
                            nc.vector.tensor_copy(stg[:], ps[:])
                            nc.sync.dma_start(dbg_aps["dbg_ps0"][:], stg[:])
                            nc.sync.dma_start(dbg_aps["dbg_pch0"][:], pch[:])
                        for j, (t, h) in enumerate(chunk):
                            rmin = 8 * max(0, t - 4 * b)
                            pcv = pch[:, 512 * j: 512 * (j + 1)].rearrange(
                                "p (c r) -> p c r", r=32)
                            nc.tensor.matmul(
                                opv[h][:, :, rmin:32],
                                vsbs[h][:, 65 * t: 65 * t + 65],
                                pcv[:, :, rmin:32],
                                start=(t == 0), stop=(t == nt - 1),
                            )
                    # normalize block b
                    for h in range(HPC):
                        if dbg and b == 0 and h == 0:
                            stg2 = ppool.tile([65, 512], f32, tag="dbgstg",
                                              name="dbgstg2")
                            nc.vector.tensor_copy(stg2[:], outp[0][:])
                            nc.sync.dma_start(dbg_aps["dbg_outp0"][:], stg2[:])
                        dnr = misc[0:1, 64 + 512: 64 + 1024]  # raw denom row
                        dn = misc[0:1, 64: 64 + 512]
                        nc.vector.tensor_copy(dnr, outp[h][64:65, :])
                        nc.vector.reciprocal_approx_fast(out=dn, in_=dnr)
                        # broadcast dn across 64 partitions: lane-0 shuffle x2
                        nc.vector.stream_shuffle(bc_sb[0:32, :],
                                                 misc[0:32, 64:576], [0] * 32)
                        nc.vector.stream_shuffle(bc_sb[32:64, :],
                                                 misc[0:32, 64:576], [0] * 32)
                        if dbg and b == 0 and h == 0:
                            nc.sync.dma_start(dbg_aps["dbg_dn0"][:], dn)
                            nc.sync.dma_start(dbg_aps["dbg_bc0"][:], bc_sb[:])
                        # outT dst: u = 256*c + r, this block is r in [32b,32b+32)
                        osl = outTs[h][:].rearrange(
                            "p (c r) -> p c r", r=256)[:, :, 32 * b: 32 * (b + 1)]
                        nc.vector.tensor_mul(osl, outp[h][0:64, :], bc_sb[:])

            if dbg:
                nc.sync.dma_start(dbg_aps["dbg_outT0"][:], outTs[0][:])
                nc.sync.dma_start(dbg_aps["dbg_outT1"][:], outTs[1][:])

            # ---------- phase 4: output projection ----------
            y_sb = big.tile([128, 4 * D], f32, tag="big")     # [p, g*1024+o]
            with tc.tile_pool(name="psy", bufs=2, space="PSUM") as psy:
                for h in range(HPC):
                    for rt in range(2):
                        g = 2 * h + rt
                        ct = ctp.tile([128, 8 * 128], bf16, tag="ct")
                        for tp in range(8):
                            # u = 256*c + r: contiguous 128-runs per (c, rt)
                            ue = 256 * (2 * tp) + 128 * rt
                            uo = 256 * (2 * tp + 1) + 128 * rt
                            # even c -> partitions [0:64), odd -> [64:128)
                            nc.scalar.activation(
                                ct[0:64, 128 * tp: 128 * tp + 128],
                                outTs[h][:, ue:ue + 128], CPY)
                            nc.vector.tensor_copy(
                                ct[64:128, 128 * tp: 128 * tp + 128],
                                outTs[h][:, uo:uo + 128])
                        for ob in range(2):
                            py = psy.tile([128, 512], f32, tag="y")
                            for tp in range(8):
                                nc.tensor.matmul(
                                    py[:],
                                    ct[:, 128 * tp: 128 * tp + 128],
                                    wo_sb[:, tp * D + ob * 512: tp * D + ob * 512 + 512],
                                    start=(tp == 0), stop=(tp == 7),
                                )
                            nc.scalar.activation(
                                y_sb[:, g * D + ob * 512: g * D + ob * 512 + 512],
                                py[:], CPY)
            nc.sync.dma_start(y.rearrange("(g p) o -> p g o", p=128),
                              y_sb[:].rearrange("p (g o) -> p g o", g=4))

    nc.compile()
    return nc


def kernel(**inputs):
    x = np.asarray(inputs["x"], dtype=np.float32)     # [1, 4096, 1024]
    Wq = np.asarray(inputs["Wq"], dtype=np.float32)
    Wk = np.asarray(inputs["Wk"], dtype=np.float32)
    Wv = np.asarray(inputs["Wv"], dtype=np.float32)
    Wo = np.asarray(inputs["Wo"], dtype=np.float32)
    # biases are structurally zero in this problem; fold anyway if nonzero
    for bn in ("bq", "bk", "bv", "bo"):
        bv_ = np.asarray(inputs.get(bn, 0.0))
        assert np.all(bv_ == 0.0), f"{bn} nonzero: unsupported"

    from concourse.bass_utils import run_bass_kernel_spmd

    if "nc" not in _CACHE:
        _CACHE["nc"] = _build_program()
    nc = _CACHE["nc"]

    bf = ml_dtypes.bfloat16
    wqT, wkT, wvT, woT, csq1, csq2, csk1, csk2, tri = _host_arrays(
        Wq, Wk, Wv, Wo)
    shared = {"wqT": wqT, "wkT": wkT, "wvT": wvT, "woT": woT,
              "cs1": csq1, "cs2": csq2, "cs3": csk1, "cs4": csk2, "tri": tri}
    xf = x.reshape(S, D)
    in_maps = []
    for i in range(NC_N):
        xTi = np.ascontiguousarray(xf[i * RPC:(i + 1) * RPC, :].T).astype(bf)
        in_maps.append(dict(shared, xT=xTi))

    trace = bool(int(os.environ.get("BASS_KERNEL_TRACE", "0")))
    res = run_bass_kernel_spmd(nc, in_maps, core_ids=list(range(NC_N)),
                               trace=trace)
    _CACHE["last_res"] = res
    if trace and res.exec_time_ns is not None:
        print(f"HW exec time: {res.exec_time_ns} ns")
        _CACHE["exec_time_ns"] = res.exec_time_ns
        _CACHE["trace"] = res.instructions_and_trace
    out = np.concatenate([res.results[i]["y"] for i in range(NC_N)], axis=0)
    return out.reshape(1, S, D).astype(np.float32)
